# revision 35
# baseline (speedup 1.0000x reference)
"""Trainium2 Bass kernel for nn_CausalPrefixAttention (8-core SPMD).

Sharding: core = b*4 + hg  (b in 0..1 batch, hg in 0..3 head-group of 2 heads).
Data parallel over batch, tensor parallel over heads: each core gets
Wq/Wkv column slices and Wo row slices for its 2 heads, computes its partial
out-projection [1024, 1024]; host sums the 4 partials per batch and adds bo.

Device-side dataflow (per core), all matmuls in f32r (fp32 with 15-bit
mantissa, full PE throughput at N>=256):
  - LayerNorm folded into weights: W' = diag(gamma) @ W (host). The per-token
    rs = 1/sqrt(var+eps) is applied in-place to x in natural layout
    (per-partition tensor_scalar). Rank-1 corrections (u = colsum W' paired
    with a (-mu*rs) row, b = beta @ W paired with a ones row) are appended as
    a K=2 augmented contraction chunk.
  - Activations transposed once via PE transposes (x^T, cx^T) so projections
    produce q^T/k^T/v^T [feature, token] directly.
  - Attention per head: sim^T = k^T.T @ q^T per 128-key tile (both heads
    concurrently via PE row groups), exp on ACT (no max subtraction: sim
    max ~7), causal-diagonal zeroing of p via a host-provided 0/1
    triangular mask multiplied on DVE, PV with ones-augmented V so the
    softmax denominator l falls out of the same matmul (row 64 of the PV
    accumulator).
  - Final: 1/l rows are broadcast across each head's 64 rows via a tiny
    selector matmul (sel.T @ 1/l) into PSUM, o^T is normalized during the
    PSUM->SBUF copy, then out = o^T.T @ Wo per 128-token tile.
  - All DMA in >=1MB-ish transfers spread over both HWDGE queues
    (sync/scalar) plus SWDGE for weights; constants (identity, masks,
    selector) are host-provided because on-device Pool ops (affine_select,
    tensor_scalar) run as slow software handlers (~14-70us each).
"""

import os
import sys

for _p in ("/opt/trn_rl_repo", "/root/.axon_site/_ro/trn_rl_repo"):
    if os.path.isdir(_p) and _p not in sys.path:
        sys.path.append(_p)

import numpy as np

import concourse.mybir as mybir
import concourse.tile as tile
from concourse import bacc
from concourse.bass_utils import run_bass_kernel_spmd

F32 = mybir.dt.float32
F32R = mybir.dt.float32r
AF = mybir.ActivationFunctionType
ALU = mybir.AluOpType

B, N, M, DIM, INNER, HEADS, DH = 2, 1024, 1024, 1024, 512, 8, 64
EPS = 1e-5
NT = N // 128      # token tiles per batch (8)
KC = DIM // 128    # contraction chunks (8)


def build_program(unroll=1, phase=2):
    nc = bacc.Bacc("TRN2", target_bir_lowering=False, debug=False)

    x_d = nc.dram_tensor("x", [N, DIM], F32R, kind="ExternalInput")
    cx_d = nc.dram_tensor("cx", [M, DIM], F32R, kind="ExternalInput")
    # in-projection weights (gamma folded), chunks [128, 9, 384]:
    # chunk c rows = contraction rows 128c..128c+127; cols 0:128 q, 128:256 k,
    # 256:384 v. Chunk 8 rows 0/1 = the (u, b) rank-1 augmentation.
    win_d = nc.dram_tensor("win", [128, KC + 1, 384], F32R, kind="ExternalInput")
    # raw context projection weights, chunks [128, 8, 256]: 0:128 k, 128:256 v
    wcx_d = nc.dram_tensor("wcx", [128, KC, 256], F32R, kind="ExternalInput")
    wo_d = nc.dram_tensor("wo", [128, DIM], F32R, kind="ExternalInput")
    sel_d = nc.dram_tensor("sel", [2, 128], F32R, kind="ExternalInput")
    tri_d = nc.dram_tensor("tri", [128, 128], F32R, kind="ExternalInput")
    idf_d = nc.dram_tensor("idf", [128, 128], F32, kind="ExternalInput")
    idr_d = nc.dram_tensor("idr", [128, 128], F32R, kind="ExternalInput")
    o_d = nc.dram_tensor("o", [N, DIM], F32, kind="ExternalOutput")

    with tile.TileContext(nc) as tc:
        for _ in range(unroll):
            _emit(nc, tc, x_d, cx_d, win_d, wcx_d, wo_d, sel_d, tri_d, idf_d, idr_d, o_d, phase)
    nc.compile()
    return nc


def _emit(nc, tc, x_d, cx_d, win_d, wcx_d, wo_d, sel_d, tri_d, idf_d, idr_d, o_d, phase=2):
    from contextlib import ExitStack

    ctx = ExitStack()
    with ctx:
        consts = ctx.enter_context(tc.tile_pool(name="consts", bufs=1))
        wpool = ctx.enter_context(tc.tile_pool(name="wpool", bufs=1))
        projp = ctx.enter_context(tc.tile_pool(name="projp", bufs=5))
        vnp = ctx.enter_context(tc.tile_pool(name="vnp", bufs=16))
        ppool = ctx.enter_context(tc.tile_pool(name="ppool", bufs=3))
        otp = ctx.enter_context(tc.tile_pool(name="otp", bufs=2))
        ostp = ctx.enter_context(tc.tile_pool(name="ostp", bufs=2))
        tiny = ctx.enter_context(tc.tile_pool(name="tiny", bufs=8))

        ident = consts.tile([128, 128], F32)
        nc.gpsimd.dma_start(out=ident, in_=idf_d[:])
        identr = consts.tile([128, 128], F32R)
        nc.gpsimd.dma_start(out=identr, in_=idr_d[:])
        eps_col = consts.tile([128, 1], F32)
        nc.vector.memset(eps_col, EPS)
        ones_col2 = consts.tile([128, 2], F32)
        nc.vector.memset(ones_col2, 1.0)
        # selector rows (f32r, host-provided, flattened to partition 0):
        # cols 0-127 = [1]*64+[0]*64, cols 128-255 = [0]*64+[1]*64.
        # sel.T @ (1/l_h) broadcasts each head's 1/l row across 64 rows.
        sel2 = consts.tile([1, 256], F32R)
        nc.gpsimd.dma_start(out=sel2, in_=sel_d[:])
        # lower-triangular-inclusive 0/1 mask (tri[j,i] = j<=i), f32r
        tri = consts.tile([128, 128], F32R)
        nc.gpsimd.dma_start(out=tri, in_=tri_d[:])

        # weights
        win = wpool.tile([128, KC + 1, 384], F32R, tag="win")
        for hw_ in range(2):
            nc.gpsimd.dma_start(
                out=win[:, :, 192 * hw_:192 * hw_ + 192],
                in_=win_d[:, :, 192 * hw_:192 * hw_ + 192])
        wcx = wpool.tile([128, KC, 256], F32R, tag="wcx")
        nc.gpsimd.dma_start(out=wcx, in_=wcx_d[:])
        wo = wpool.tile([128, DIM], F32R, tag="wo")
        nc.gpsimd.dma_start(out=wo, in_=wo_d[:])

        # augmentation rows: row0 = -mu*rs per token, row1 = ones
        # (memset both rows to 1; row0 gets overwritten by the stats copies)
        stats_r = consts.tile([2, N], F32R)
        ones_rows = consts.tile([2, N], F32)
        nc.vector.memset(ones_rows, 1.0)
        nc.vector.tensor_copy(out=stats_r, in_=ones_rows)

        kcxT = projp.tile([128, M], F32R, tag="proj", name="kcxT")
        vcxT = projp.tile([128, M], F32R, tag="proj", name="vcxT")
        qT = projp.tile([128, N], F32R, tag="proj", name="qT")
        kinT = projp.tile([128, N], F32R, tag="proj", name="kinT")
        vinT = projp.tile([128, N], F32R, tag="proj", name="vinT")
        vn = [None] * 16

        phase_a = ExitStack()
        with phase_a:
            natcx = phase_a.enter_context(tc.tile_pool(name="natcx", bufs=1))
            natx = phase_a.enter_context(tc.tile_pool(name="natx", bufs=1))
            tposed = phase_a.enter_context(tc.tile_pool(name="tposed", bufs=8))
            psA = phase_a.enter_context(
                tc.tile_pool(name="psA", bufs=1, space="PSUM"))

            def transpose_128(dsts, srcs, ident_, dtype, copy_engines):
                # dsts: list of [128, 1024] tiles (chunk-major); srcs: list of
                # 8 natural [128, 1024] tiles. PE-transpose 128x128 blocks in
                # groups of 4 into one PSUM bank, alternate copy engines.
                ci = 0
                for c, dst in enumerate(dsts):
                    for tq in range(2):
                        ps = psA.tile([128, 512], dtype, tag="tps", bufs=3,
                                      name="tps")
                        for k in range(4):
                            t = tq * 4 + k
                            nc.tensor.transpose(
                                ps[:, k * 128:(k + 1) * 128],
                                srcs[t][:, c * 128:(c + 1) * 128], ident_)
                        eng = copy_engines[ci % len(copy_engines)]
                        ci += 1
                        if eng == "act":
                            nc.scalar.copy(
                                out=dst[:, tq * 512:(tq + 1) * 512], in_=ps)
                        else:
                            nc.vector.tensor_copy(
                                out=dst[:, tq * 512:(tq + 1) * 512], in_=ps)

            # ---- context: load, transpose, cx projections ----
            cxnat_t = natcx.tile([128, NT, DIM], F32R, tag="nat", name="cxnat")
            cx_r = cx_d.rearrange("(t p) d -> p t d", p=128)
            for hf in range(NT):
                eng = nc.scalar if hf % 2 == 0 else nc.sync
                eng.dma_start(out=cxnat_t[:, hf:hf + 1, :],
                              in_=cx_r[:, hf:hf + 1, :])
            cx_nat = [cxnat_t[:, t, :] for t in range(NT)]
            # ---- x: load + stats (DVE/ACT) + in-place rs scale (Pool) ----
            xnat_t = natx.tile([128, NT, DIM], F32R, tag="nat", name="xnat")
            x_r = x_d.rearrange("(t p) d -> p t d", p=128)
            for hf in range(NT):
                eng = nc.sync if hf % 2 == 0 else nc.scalar
                eng.dma_start(out=xnat_t[:, hf:hf + 1, :],
                              in_=x_r[:, hf:hf + 1, :])
            x_nat = [xnat_t[:, t, :] for t in range(NT)]
            stats4 = []
            for t in range(NT):
                xt = x_nat[t]
                s4 = tiny.tile([128, 4], F32, tag="s4", name=f"s4_{t}")
                stats4.append(s4)
                bst = tiny.tile([128, 2, 6], F32, tag="bst", name="bst")
                for half in range(2):
                    nc.vector.bn_stats(
                        out=bst[:, half, :],
                        in_=xt[:, half * 512:(half + 1) * 512])
                mv = tiny.tile([128, 2], F32, tag="mv", name="mv")
                nc.vector.bn_aggr(out=mv, in_=bst)
                # rs = 1/sqrt(var+eps) -> s4 col2
                std = tiny.tile([128, 1], F32, tag="c6", name="std")
                nc.scalar.activation(
                    out=std, in_=mv[:, 1:2], func=AF.Sqrt, bias=eps_col)
                nc.vector.reciprocal(out=s4[:, 2:3], in_=std)
                # negmurs = -mean*rs -> s4 col0
                nc.vector.scalar_tensor_tensor(
                    out=s4[:, 0:1], in0=mv[:, 0:1], scalar=-1.0,
                    in1=s4[:, 2:3], op0=ALU.mult, op1=ALU.mult)
                # x <- x * rs (in place, natural layout)
                nc.vector.tensor_scalar(
                    out=xt, in0=xt, scalar1=s4[:, 2:3], scalar2=None,
                    op0=ALU.mult)

            if phase == 0:
                o_r0 = o_d.rearrange("(t p) d -> p t d", p=128).bitcast(F32R)
                for hf in range(2):
                    eng = nc.sync if hf % 2 == 0 else nc.scalar
                    eng.dma_start(out=o_r0[:, 4 * hf:4 * hf + 4, :],
                                  in_=xnat_t[:, 4 * hf:4 * hf + 4, :])
                return
            cxT = [tposed.tile([128, M], F32R, tag="tp", name=f"cxT{c}")
                   for c in range(KC)]
            transpose_128(cxT, cx_nat, identr, F32R, ("dve", "act"))

            for pj, dst in ((0, kcxT), (1, vcxT)):
                for g in range(2):
                    sp = slice(g * 512, (g + 1) * 512)
                    ps = psA.tile([128, 512], F32, tag="pps", bufs=3,
                                  name="pps")
                    for c in range(KC):
                        nc.tensor.matmul(
                            ps, wcx[:, c, pj * 128:(pj + 1) * 128],
                            cxT[c][:, sp],
                            start=(c == 0), stop=(c == KC - 1))
                    if g == 0:
                        nc.vector.tensor_copy(out=dst[:, sp], in_=ps)
                    else:
                        nc.scalar.copy(out=dst[:, sp], in_=ps)

            # v_nat for the context half: [j, 130] with ones cols at 64 / 129
            def v_transpose(src, base):
                for j in range(8):
                    v_t = vnp.tile([128, 130], F32R, tag="vn",
                                   name=f"vn{base + j}")
                    vn[base + j] = v_t
                    ps = psA.tile([128, 512], F32R, tag="tpsr", bufs=2,
                                  name="tpsr")
                    nc.tensor.transpose(
                        ps[:, 0:128], src[:, j * 128:(j + 1) * 128], identr)
                    nc.vector.tensor_copy(
                        out=v_t.rearrange("p (a b) -> p a b", b=65)[:, :, 0:64],
                        in_=ps[:, 0:128].rearrange("p (a b) -> p a b", b=64))
                    nc.vector.tensor_copy(
                        out=v_t.rearrange("p (a b) -> p a b", b=65)[:, :, 64:65],
                        in_=ones_col2.rearrange("p (a b) -> p a b", b=1))

            v_transpose(vcxT, 0)

            # ---- x transposes (reuse cxT slots) + stats row ----
            for t in range(NT):
                ps = psA.tile([128, 512], F32, tag="tps", bufs=3, name="tps")
                nc.tensor.transpose(ps[0:4, 0:128], stats4[t], ident)
                nc.vector.tensor_copy(
                    out=stats_r[0:1, t * 128:(t + 1) * 128], in_=ps[0:1, 0:128])
            xT = [tposed.tile([128, N], F32R, tag="tp", name=f"xT{c}")
                  for c in range(KC)]
            transpose_128(xT, x_nat, identr, F32R, ("dve", "act"))

            # ---- input projections (q first so attention can start) ----
            for pj, dst in ((0, qT), (2, vinT), (1, kinT)):
                wsl = slice(pj * 128, (pj + 1) * 128)
                for g in range(2):
                    sp = slice(g * 512, (g + 1) * 512)
                    ps = psA.tile([128, 512], F32, tag="pps", bufs=3,
                                  name="pps")
                    for c in range(KC):
                        nc.tensor.matmul(
                            ps, win[:, c, wsl], xT[c][:, sp],
                            start=(c == 0), stop=False)
                    nc.tensor.matmul(
                        ps, win[0:2, KC, wsl], stats_r[:, sp],
                        start=False, stop=True)
                    if g == 0:
                        nc.vector.tensor_copy(out=dst[:, sp], in_=ps)
                    else:
                        nc.scalar.copy(out=dst[:, sp], in_=ps)

            v_transpose(vinT, 8)

            if phase == 1:
                for t, src_t in enumerate((qT, kinT, vinT, kcxT, vcxT,
                                           qT, kinT, vinT)):
                    nc.sync.dma_start(
                        out=o_d[t * 128:(t + 1) * 128, :].bitcast(F32R),
                        in_=src_t)
                return

        # ---- attention + final projection ----
        with tc.tile_pool(name="psSim", bufs=1, space="PSUM") as psS, \
             tc.tile_pool(name="psO", bufs=1, space="PSUM") as psO, \
             tc.tile_pool(name="psF", bufs=1, space="PSUM") as psF:
            for g in range(2):
                # j order: cx0..cx6, in0.., cx7 (start/stop on full spans)
                j_list = [("cx", j) for j in range(7)]
                j_list += [("in", j) for j in range(4 * g + 4)]
                j_list.append(("cx", 7))
                n_j = len(j_list)
                o_ps = [psO.tile([128, 512], F32, tag=f"o{h}", name=f"ops{h}")
                        for h in (0, 1)]
                for idx, (src, j) in enumerate(j_list):
                    if src == "cx":
                        kT, jg, off = kcxT, j, 0
                    else:
                        kT, jg = kinT, 8 + j
                        off = max(0, 128 * (j - 4 * g))
                    diag = src == "in" and j >= 4 * g
                    p_t = [None, None]
                    for h in (0, 1):
                        hsl = slice(64 * h, 64 * h + 64)
                        ps = psS.tile([128, 512], F32, tag=f"sim{h}", bufs=2,
                                      name=f"sim{h}")
                        nc.tensor.matmul(
                            ps[:, off:512],
                            kT[hsl, j * 128:(j + 1) * 128],
                            qT[hsl, g * 512 + off:(g + 1) * 512],
                            start=True, stop=True)
                        p_t[h] = ppool.tile([128, 512], F32R, tag=f"p{h}",
                                            name=f"p{h}")
                        nc.scalar.activation(
                            out=p_t[h][:, off:512], in_=ps[:, off:512],
                            func=AF.Exp)
                        if diag:
                            nc.vector.tensor_tensor(
                                out=p_t[h][:, off:off + 128],
                                in0=p_t[h][:, off:off + 128],
                                in1=tri, op=ALU.mult)
                    for h in (0, 1):
                        nc.tensor.matmul(
                            o_ps[h][0:65, off:512],
                            vn[jg][:, 65 * h:65 * h + 65],
                            p_t[h][:, off:512],
                            start=(idx == 0), stop=(idx == n_j - 1))

                # l rows -> 1/l -> broadcast to [128,512] via sel-matrix
                # matmul (rows 0-63 get 1/l0, rows 64-127 get 1/l1), then
                # normalize o while copying PSUM->SBUF.
                lrec = [tiny.tile([1, 512], F32R, tag=f"lr{h}", bufs=2,
                                  name=f"lr{h}") for h in (0, 1)]
                with nc.allow_low_precision(reason="1/l in f32r is plenty"):
                    for h in (0, 1):
                        nc.vector.tensor_copy(out=lrec[h],
                                              in_=o_ps[h][64:65, :])
                        nc.vector.reciprocal(out=lrec[h], in_=lrec[h])
                lbc_ps = psF.tile([128, 512], F32, tag="fin0", bufs=1,
                                  name="lbc")
                for h in (0, 1):
                    nc.tensor.matmul(lbc_ps, sel2[:, 128 * h:128 * h + 128],
                                     lrec[h], start=(h == 0), stop=(h == 1))
                lbc = tiny.tile([128, 512], F32, tag="lbc", bufs=2, name="lbc")
                nc.vector.tensor_copy(out=lbc, in_=lbc_ps)
                # normalized merged head outputs (f32r for the final matmul)
                oT = otp.tile([128, 512], F32R, tag="oT")
                for h in (0, 1):
                    nc.vector.tensor_tensor(
                        out=oT[64 * h:64 * h + 64, :], in0=o_ps[h][0:64, :],
                        in1=lbc[64 * h:64 * h + 64, :], op=ALU.mult)

                o_r = o_d.rearrange("(t p) d -> p t d", p=128)
                for tp in range(2):
                    ost = ostp.tile([128, 2, DIM], F32, tag="ost")
                    for ti in range(2):
                        t = tp * 2 + ti
                        for half in range(2):
                            wsp = slice(half * 512, (half + 1) * 512)
                            fp = psF.tile([128, 512], F32, tag=f"fin{half}",
                                          bufs=1, name=f"fin{half}")
                            nc.tensor.matmul(
                                fp, oT[:, t * 128:(t + 1) * 128], wo[:, wsp],
                                start=True, stop=True)
                            if half == 0:
                                nc.vector.tensor_copy(
                                    out=ost[:, ti, wsp], in_=fp)
                            else:
                                nc.scalar.copy(out=ost[:, ti, wsp], in_=fp)
                    eng = nc.sync if tp % 2 == 0 else nc.scalar
                    eng.dma_start(
                        out=o_r[:, g * 4 + tp * 2:g * 4 + tp * 2 + 2, :],
                        in_=ost)


_NC_CACHE = None


def _get_nc():
    global _NC_CACHE
    if _NC_CACHE is None:
        _NC_CACHE = build_program()
    return _NC_CACHE


def make_in_maps(x, context, gamma, beta, Wq, Wkv, Wo, bo):
    x = np.asarray(x, np.float32)
    context = np.asarray(context, np.float32)
    gamma = np.asarray(gamma, np.float32)
    beta = np.asarray(beta, np.float32)
    Wq = np.asarray(Wq, np.float32)
    Wkv = np.asarray(Wkv, np.float32)
    Wo = np.asarray(Wo, np.float32)

    s = DH ** -0.5
    in_maps = []
    for core in range(8):
        b, hg = divmod(core, 4)
        cols = slice(128 * hg, 128 * hg + 128)
        wq = Wq[:, cols] * gamma[:, None] * s
        uq = wq.sum(0)
        bq = beta @ Wq[:, cols] * s
        wk = Wkv[:, :INNER][:, cols] * gamma[:, None]
        uk = wk.sum(0)
        bk = beta @ Wkv[:, :INNER][:, cols]
        wv = Wkv[:, INNER:][:, cols] * gamma[:, None]
        uv = wv.sum(0)
        bv = beta @ Wkv[:, INNER:][:, cols]

        win = np.zeros((128, KC + 1, 384), np.float32)
        for c in range(KC):
            rows = slice(128 * c, 128 * c + 128)
            win[:, c, 0:128] = wq[rows]
            win[:, c, 128:256] = wk[rows]
            win[:, c, 256:384] = wv[rows]
        win[0, KC, 0:128] = uq
        win[1, KC, 0:128] = bq
        win[0, KC, 128:256] = uk
        win[1, KC, 128:256] = bk
        win[0, KC, 256:384] = uv
        win[1, KC, 256:384] = bv

        wcx = np.zeros((128, KC, 256), np.float32)
        for c in range(KC):
            rows = slice(128 * c, 128 * c + 128)
            wcx[:, c, 0:128] = Wkv[:, :INNER][rows, cols]
            wcx[:, c, 128:256] = Wkv[:, INNER:][rows, cols]

        sel = np.zeros((2, 128), np.float32)
        sel[0, 0:64] = 1.0
        sel[1, 64:128] = 1.0
        tri = np.tril(np.ones((128, 128), np.float32)).T
        idm = np.eye(128, dtype=np.float32)
        in_maps.append({
            "idf": idm,
            "idr": idm,
            "sel": sel,
            "tri": tri,
            "x": np.ascontiguousarray(x[b]),
            "cx": np.ascontiguousarray(context[b]),
            "win": win,
            "wcx": wcx,
            "wo": np.ascontiguousarray(Wo[cols, :]),
        })
    return in_maps


def assemble(results, bo):
    bo = np.asarray(bo, np.float32)
    out = np.zeros((B, N, DIM), np.float32)
    for core in range(8):
        b = core // 4
        out[b] += results[core]["o"]
    out += bo[None, None, :]
    return out


def kernel(x, context, gamma, beta, Wq, Wkv, Wo, bo):
    nc = _get_nc()
    in_maps = make_in_maps(x, context, gamma, beta, Wq, Wkv, Wo, bo)
    res = run_bass_kernel_spmd(nc, in_maps, list(range(8)))
    return assemble(res.results, bo)
